# revision 6
# baseline (speedup 1.0000x reference)
#!/usr/bin/env python3
"""GroupedQueryAttention Trainium2 kernel, tensor-parallel over heads on 8
NeuronCores.

Reference model: B=2, S=2048, H=4096, NH=32 query heads, NKV=8 kv heads,
HD=128, RoPE base 5e5, softmax attention, o-proj.

Sharding: core c owns kv head c and query heads 4c..4c+3 (groups stay
aligned).  Wq/Wo sharded by query head, Wk/Wv by kv head.  Each core
computes a rank-512 slice of the o-proj contraction; the host sums the 8
partial outputs (the all-reduce of row-parallel TP done on host at gather
time).

On-core dataflow (per batch b):
  P: QKV projections.  X^T streamed h-major; Wq/Wk/Wv slices resident.
     PSUM accumulates over 32 h-tiles; RoPE applied on DVE straight out of
     PSUM; V^T transposed back to token-major via PE transposes (bf16).
  A: attention per query head.  Scores computed transposed (j on
     partitions) so softmax denominators come from a ones-matmul; exp on
     ACT (fused 1/sqrt(128) scale) writes bf16 P-tilde; P@V accumulates in
     PSUM over j-tiles; normalization multiplies by a PE-broadcast
     reciprocal on the way to SBUF.
  W: o-proj partial.  O^T slices are the stationary operand directly; Wo^T
     streamed in 1 MB chunks.

All matmul inputs are float32r (full-rate PE path) except P-tilde/V which
are bf16.  Weights/activations never leave fp32 precision elsewhere.
"""
import sys

for _p in ("/opt/trn_rl_repo",):
    if _p not in sys.path:
        sys.path.insert(0, _p)

import numpy as np

import concourse.bacc as bacc
import concourse.mybir as mybir
from concourse import tile
from concourse.bass_utils import run_bass_kernel_spmd

B, S, H = 2, 2048, 4096
NH, NKV, HD = 32, 8, 128
NCORES = 8
QH_PER_CORE = NH // NCORES          # 4 query heads / core
QD = QH_PER_CORE * HD               # 512 q dims / core
ROPE_BASE = 500000.0
T = B * S                           # 4096 tokens
TCH = 512                           # token chunk (proj N, attn i-chunk)
NTCH = S // TCH                     # 4 chunks per batch
HT = H // 128                       # 32 h-tiles
JT = S // 128                       # 16 j-tiles per batch
SCALE = 1.0 / np.sqrt(HD)

F32 = mybir.dt.float32
F32R = mybir.dt.float32r
BF16 = mybir.dt.bfloat16


def _build_nc():
    nc = bacc.Bacc("TRN2", target_bir_lowering=False, debug=False)
    xt = nc.dram_tensor("xt", [H, T], F32R, kind="ExternalInput").ap()
    wq = nc.dram_tensor("wq", [H, QD], F32R, kind="ExternalInput").ap()
    wk = nc.dram_tensor("wk", [H, HD], F32R, kind="ExternalInput").ap()
    wv = nc.dram_tensor("wv", [H, HD], F32R, kind="ExternalInput").ap()
    wo = nc.dram_tensor("wo", [QD, H], F32R, kind="ExternalInput").ap()
    cosx = nc.dram_tensor("cosx", [HD, S], F32, kind="ExternalInput").ap()
    ssin = nc.dram_tensor("ssin", [HD, S], F32, kind="ExternalInput").ap()
    ident = nc.dram_tensor("ident", [128, 128], F32, kind="ExternalInput").ap()
    ones_bf = nc.dram_tensor("ones_bf", [128, 1], BF16, kind="ExternalInput").ap()
    ones_fr = nc.dram_tensor("ones_fr", [1, 128], F32R, kind="ExternalInput").ap()
    out = nc.dram_tensor("out_part", [T, H], F32, kind="ExternalOutput").ap()

    with tile.TileContext(nc) as tc, \
         nc.allow_low_precision(reason="fp32r matmul inputs; bf16 attn probs"):
        with tc.tile_pool(name="persist", bufs=1) as persist, \
             tc.tile_pool(name="batch", bufs=1) as bpool:
            cos_sb = persist.tile([HD, S], F32)
            nc.sync.dma_start(cos_sb[:], cosx[:])
            ssin_sb = persist.tile([HD, S], F32)
            nc.sync.dma_start(ssin_sb[:], ssin[:])
            ident_sb = persist.tile([128, 128], F32)
            nc.sync.dma_start(ident_sb[:], ident[:])
            onesb_sb = persist.tile([128, 1], BF16)
            nc.sync.dma_start(onesb_sb[:], ones_bf[:])
            onesr_sb = persist.tile([1, 128], F32R)
            nc.sync.dma_start(onesr_sb[:], ones_fr[:])

            for b in range(B):
                t0 = b * S
                # per-batch activation stores (tags shared across batches)
                qt_sb = [
                    bpool.tile([128, S], F32R, name=f"qt{qh}_b{b}", tag=f"qt{qh}")
                    for qh in range(QH_PER_CORE)
                ]
                kt_sb = bpool.tile([128, S], F32R, name=f"kt_b{b}", tag="kt")
                v_sb = bpool.tile([128, JT, 128], BF16, name=f"v_b{b}", tag="v")
                ot_sb = [
                    bpool.tile([128, S], F32R, name=f"ot{qh}_b{b}", tag=f"ot{qh}")
                    for qh in range(QH_PER_CORE)
                ]

                # ---------------- P: QKV projections ----------------
                with tc.tile_pool(name="pw", bufs=1) as pw, \
                     tc.tile_pool(name="px", bufs=3) as px, \
                     tc.tile_pool(name="pt", bufs=2) as ptmp, \
                     tc.tile_pool(name="pps", bufs=1, space="PSUM") as pps, \
                     tc.tile_pool(name="vps", bufs=2, space="PSUM") as vps:
                    wq_sb = pw.tile([128, HT, QD], F32R)
                    nc.sync.dma_start(
                        wq_sb[:], wq.rearrange("(a p) q -> p a q", p=128))

                    for tch in range(NTCH):
                        tc0 = t0 + tch * TCH
                        q_ps = [
                            pps.tile([128, TCH], F32, name=f"qps{i}", tag=f"qps{i}")
                            for i in range(QH_PER_CORE)
                        ]
                        k_ps = pps.tile([128, TCH], F32, name="kps", tag="kps")
                        v_ps = pps.tile([128, TCH], F32, name="vps0", tag="vps0")
                        for hg in range(HT // 2):
                            x_t = px.tile([128, 2, TCH], F32R, name="xs", tag="xs")
                            nc.sync.dma_start(
                                x_t[:],
                                xt[hg * 256:(hg + 1) * 256, tc0:tc0 + TCH]
                                .rearrange("(a p) t -> p a t", p=128))
                            wk_t = px.tile([128, 2, HD], F32R, name="wks", tag="wks")
                            nc.sync.dma_start(
                                wk_t[:],
                                wk[hg * 256:(hg + 1) * 256, :]
                                .rearrange("(a p) q -> p a q", p=128))
                            wv_t = px.tile([128, 2, HD], F32R, name="wvs", tag="wvs")
                            nc.sync.dma_start(
                                wv_t[:],
                                wv[hg * 256:(hg + 1) * 256, :]
                                .rearrange("(a p) q -> p a q", p=128))
                            for hi in range(2):
                                h = hg * 2 + hi
                                first, last = h == 0, h == HT - 1
                                for qd in range(QH_PER_CORE):
                                    nc.tensor.matmul(
                                        q_ps[qd][:],
                                        wq_sb[:, h, qd * 128:(qd + 1) * 128],
                                        x_t[:, hi, :], start=first, stop=last)
                                nc.tensor.matmul(k_ps[:], wk_t[:, hi, :],
                                                 x_t[:, hi, :], start=first,
                                                 stop=last)
                                nc.tensor.matmul(v_ps[:], wv_t[:, hi, :],
                                                 x_t[:, hi, :], start=first,
                                                 stop=last)
                        # RoPE (DVE, reads PSUM) → fp32r activation stores
                        csl = slice(tch * TCH, (tch + 1) * TCH)
                        for qd in range(QH_PER_CORE):
                            tA = ptmp.tile([128, TCH], F32, name="ropeA", tag="ropeA")
                            nc.vector.tensor_tensor(
                                tA[:], q_ps[qd][:], cos_sb[:, csl],
                                mybir.AluOpType.mult)
                            tBq = ptmp.tile([128, TCH], F32, name="ropeB", tag="ropeB")
                            nc.vector.tensor_tensor(
                                tBq[0:64, :], q_ps[qd][64:128, :],
                                ssin_sb[0:64, csl], mybir.AluOpType.mult)
                            nc.vector.tensor_tensor(
                                tBq[64:128, :], q_ps[qd][0:64, :],
                                ssin_sb[64:128, csl], mybir.AluOpType.mult)
                            nc.vector.tensor_tensor(
                                qt_sb[qd][:, csl], tA[:], tBq[:],
                                mybir.AluOpType.add)
                        tA = ptmp.tile([128, TCH], F32, name="ropeA", tag="ropeA")
                        nc.vector.tensor_tensor(
                            tA[:], k_ps[:], cos_sb[:, csl], mybir.AluOpType.mult)
                        tBk = ptmp.tile([128, TCH], F32, name="ropeB", tag="ropeB")
                        nc.vector.tensor_tensor(
                            tBk[0:64, :], k_ps[64:128, :], ssin_sb[0:64, csl],
                            mybir.AluOpType.mult)
                        nc.vector.tensor_tensor(
                            tBk[64:128, :], k_ps[0:64, :], ssin_sb[64:128, csl],
                            mybir.AluOpType.mult)
                        nc.vector.tensor_tensor(
                            kt_sb[:, csl], tA[:], tBk[:], mybir.AluOpType.add)
                        # V: PSUM → SBUF, then PE-transpose to token-major bf16
                        vraw = ptmp.tile([128, TCH], F32, name="vraw", tag="vraw")
                        nc.scalar.copy(vraw[:], v_ps[:])
                        for tt in range(TCH // 128):
                            vt_ps = vps.tile([128, 128], F32, name="vtp", tag="vtp")
                            nc.tensor.transpose(
                                vt_ps[:], vraw[:, tt * 128:(tt + 1) * 128],
                                ident_sb[:])
                            nc.vector.tensor_copy(
                                v_sb[:, tch * 4 + tt, :], vt_ps[:])

                # ---------------- A: attention ----------------
                with tc.tile_pool(name="ap", bufs=2) as apool, \
                     tc.tile_pool(name="an", bufs=4) as anorm, \
                     tc.tile_pool(name="sps", bufs=2, space="PSUM") as sps, \
                     tc.tile_pool(name="ops", bufs=2, space="PSUM") as ops_, \
                     tc.tile_pool(name="dps", bufs=2, space="PSUM") as dps:
                    for qh in range(QH_PER_CORE):
                        # software pipeline: scores(i+1) issued before PV(i)
                        stage = []  # (ich, p_sb, den_ps)
                        for ich in range(NTCH + 1):
                            if ich < NTCH:
                                isl = slice(ich * TCH, (ich + 1) * TCH)
                                p_sb = apool.tile([128, JT, TCH], BF16,
                                                  name="ptil", tag="ptil")
                                den_ps = dps.tile([1, TCH], F32, name="den",
                                                  tag="den")
                                for jt in range(JT):
                                    st_ps = sps.tile([128, TCH], F32,
                                                     name="st", tag="st")
                                    nc.tensor.matmul(
                                        st_ps[:],
                                        kt_sb[:, jt * 128:(jt + 1) * 128],
                                        qt_sb[qh][:, isl],
                                        start=True, stop=True)
                                    nc.scalar.activation(
                                        p_sb[:, jt, :], st_ps[:],
                                        mybir.ActivationFunctionType.Exp,
                                        scale=SCALE)
                                    nc.tensor.matmul(
                                        den_ps[:], onesb_sb[:], p_sb[:, jt, :],
                                        start=(jt == 0), stop=(jt == JT - 1))
                                stage.append((ich, p_sb, den_ps))
                            if (ich >= 1 and stage) or ich == NTCH:
                                cich, p_sb, den_ps = stage.pop(0)
                                isl = slice(cich * TCH, (cich + 1) * TCH)
                                rec = anorm.tile([1, TCH], F32R, name="rec",
                                                 tag="rec")
                                nc.vector.reciprocal(rec[:], den_ps[:])
                                bc_ps = dps.tile([128, TCH], F32, name="bc",
                                                 tag="bc")
                                nc.tensor.matmul(bc_ps[:], onesr_sb[:], rec[:],
                                                 start=True, stop=True)
                                bc_sb = anorm.tile([128, TCH], F32, name="bcs",
                                                   tag="bcs")
                                nc.scalar.copy(bc_sb[:], bc_ps[:])
                                o_ps = ops_.tile([128, TCH], F32, name="ops0",
                                                 tag="ops0")
                                for jt in range(JT):
                                    nc.tensor.matmul(
                                        o_ps[:], v_sb[:, jt, :], p_sb[:, jt, :],
                                        start=(jt == 0), stop=(jt == JT - 1))
                                nc.vector.tensor_tensor(
                                    ot_sb[qh][:, isl], o_ps[:], bc_sb[:],
                                    mybir.AluOpType.mult)

                # ---------------- W: o-proj partial ----------------
                with tc.tile_pool(name="wo", bufs=2) as wop, \
                     tc.tile_pool(name="oc", bufs=4) as ocp, \
                     tc.tile_pool(name="wps", bufs=4, space="PSUM") as wps:
                    for hch in range(H // TCH):
                        hsl = slice(hch * TCH, (hch + 1) * TCH)
                        wo_t = wop.tile([128, QH_PER_CORE, TCH], F32R,
                                        name="wot", tag="wot")
                        nc.sync.dma_start(
                            wo_t[:],
                            wo[:, hsl].rearrange("(a p) hh -> p a hh", p=128))
                        for tt in range(S // 128):
                            o_ps = wps.tile([128, TCH], F32, name="wops",
                                            tag="wops")
                            for od in range(QH_PER_CORE):
                                nc.tensor.matmul(
                                    o_ps[:],
                                    ot_sb[od][:, tt * 128:(tt + 1) * 128],
                                    wo_t[:, od, :],
                                    start=(od == 0), stop=(od == QH_PER_CORE - 1))
                            o_sb = ocp.tile([128, TCH], F32, name="osb",
                                            tag="osb")
                            nc.vector.tensor_copy(o_sb[:], o_ps[:])
                            nc.sync.dma_start(
                                out[t0 + tt * 128:t0 + (tt + 1) * 128, hsl],
                                o_sb[:])
    nc.finalize()
    return nc


_NC_CACHE = None


def _get_nc():
    global _NC_CACHE
    if _NC_CACHE is None:
        _NC_CACHE = _build_nc()
    return _NC_CACHE


def _host_tables():
    inv = 1.0 / (ROPE_BASE ** (np.arange(0, HD, 2, dtype=np.float64) / HD))
    t = np.arange(S, dtype=np.float64)
    freqs = np.outer(t, inv)                      # [S, 64]
    emb = np.concatenate([freqs, freqs], axis=1)  # [S, 128]
    cos = np.cos(emb).astype(np.float32).T.copy()   # [128, S]
    sin = np.sin(emb).astype(np.float32).T.copy()
    ssin = sin.copy()
    ssin[0:64, :] *= -1.0
    return np.ascontiguousarray(cos), np.ascontiguousarray(ssin)


def kernel(hidden_states, Wq, Wk, Wv, Wo, trace=False):
    hs = np.asarray(hidden_states, dtype=np.float32)
    Wq = np.asarray(Wq, dtype=np.float32)
    Wk = np.asarray(Wk, dtype=np.float32)
    Wv = np.asarray(Wv, dtype=np.float32)
    Wo = np.asarray(Wo, dtype=np.float32)

    import ml_dtypes
    xt = np.ascontiguousarray(hs.reshape(T, H).T)          # [H, T]
    cos, ssin = _host_tables()
    ident = np.eye(128, dtype=np.float32)
    ones_bf = np.ones((128, 1), dtype=ml_dtypes.bfloat16)
    ones_fr = np.ones((1, 128), dtype=np.float32)

    in_maps = []
    for c in range(NCORES):
        in_maps.append({
            "xt": xt,
            "wq": np.ascontiguousarray(Wq[c * QD:(c + 1) * QD, :].T),
            "wk": np.ascontiguousarray(Wk[c * HD:(c + 1) * HD, :].T),
            "wv": np.ascontiguousarray(Wv[c * HD:(c + 1) * HD, :].T),
            "wo": np.ascontiguousarray(Wo[:, c * QD:(c + 1) * QD].T),
            "cosx": cos,
            "ssin": ssin,
            "ident": ident,
            "ones_bf": ones_bf,
            "ones_fr": ones_fr,
        })

    nc = _get_nc()
    res = run_bass_kernel_spmd(nc, in_maps, list(range(NCORES)), trace=trace)
    acc = np.zeros((T, H), dtype=np.float32)
    for c in range(NCORES):
        acc += res.results[c]["out_part"]
    out = acc.reshape(B, S, H)
    if trace:
        return out, res
    return out


# revision 11
# speedup vs baseline: 1.0105x; 1.0105x over previous
#!/usr/bin/env python3
"""GroupedQueryAttention Trainium2 kernel, tensor-parallel over heads on 8
NeuronCores.

Reference model: B=2, S=2048, H=4096, NH=32 query heads, NKV=8 kv heads,
HD=128, RoPE base 5e5, softmax attention, o-proj.

Sharding: core c owns kv head c and query heads 4c..4c+3 (groups stay
aligned).  Wq/Wo sharded by query head, Wk/Wv by kv head.  Each core
computes a rank-512 slice of the o-proj contraction; the host sums the 8
partial outputs (the all-reduce of row-parallel TP done on host at gather
time).

On-core dataflow (per batch b):
  P: QKV projections.  X^T streamed h-major; Wq/Wk/Wv slices resident.
     PSUM accumulates over 32 h-tiles; RoPE applied on DVE straight out of
     PSUM; V^T transposed back to token-major via PE transposes (bf16).
  A: attention per query head.  Scores computed transposed (j on
     partitions) so softmax denominators come from a ones-matmul; exp on
     ACT (fused 1/sqrt(128) scale) writes bf16 P-tilde; P@V accumulates in
     PSUM over j-tiles; normalization multiplies by a PE-broadcast
     reciprocal on the way to SBUF.
  W: o-proj partial.  O^T slices are the stationary operand directly; Wo^T
     streamed in 1 MB chunks.

All matmul inputs are float32r (full-rate PE path) except P-tilde/V which
are bf16.  Weights/activations never leave fp32 precision elsewhere.
"""
import sys

for _p in ("/opt/trn_rl_repo",):
    if _p not in sys.path:
        sys.path.insert(0, _p)

import numpy as np

import concourse.bacc as bacc
import concourse.mybir as mybir
from concourse import tile
from concourse.bass_utils import run_bass_kernel_spmd

B, S, H = 2, 2048, 4096
NH, NKV, HD = 32, 8, 128
NCORES = 8
QH_PER_CORE = NH // NCORES          # 4 query heads / core
QD = QH_PER_CORE * HD               # 512 q dims / core
ROPE_BASE = 500000.0
T = B * S                           # 4096 tokens
TCH = 512                           # token chunk (proj N, attn i-chunk)
NTCH = S // TCH                     # 4 chunks per batch
HT = H // 128                       # 32 h-tiles
JT = S // 128                       # 16 j-tiles per batch
SCALE = 1.0 / np.sqrt(HD)

F32 = mybir.dt.float32
F32R = mybir.dt.float32r
BF16 = mybir.dt.bfloat16


def _build_nc():
    nc = bacc.Bacc("TRN2", target_bir_lowering=False, debug=False)
    xt = nc.dram_tensor("xt", [H, T], F32R, kind="ExternalInput").ap()
    wq = nc.dram_tensor("wq", [H, QD], F32R, kind="ExternalInput").ap()
    wk = nc.dram_tensor("wk", [H, HD], F32R, kind="ExternalInput").ap()
    wv = nc.dram_tensor("wv", [H, HD], F32R, kind="ExternalInput").ap()
    wo = nc.dram_tensor("wo", [QD, H], F32R, kind="ExternalInput").ap()
    cosx = nc.dram_tensor("cosx", [HD, S], F32, kind="ExternalInput").ap()
    ssin = nc.dram_tensor("ssin", [HD, S], F32, kind="ExternalInput").ap()
    ident = nc.dram_tensor("ident", [128, 128], F32, kind="ExternalInput").ap()
    ones_bf = nc.dram_tensor("ones_bf", [128, 1], BF16, kind="ExternalInput").ap()
    ones_fr = nc.dram_tensor("ones_fr", [1, 128], F32R, kind="ExternalInput").ap()
    out = nc.dram_tensor("out_part", [T, H], F32, kind="ExternalOutput").ap()

    with tile.TileContext(nc) as tc, \
         nc.allow_low_precision(reason="fp32r matmul inputs; bf16 attn probs"):
        with tc.tile_pool(name="persist", bufs=1) as persist, \
             tc.tile_pool(name="batch", bufs=1) as bpool:
            cos_sb = persist.tile([HD, S], F32)
            nc.sync.dma_start(cos_sb[:], cosx[:])
            ssin_sb = persist.tile([HD, S], F32)
            nc.sync.dma_start(ssin_sb[:], ssin[:])
            ident_sb = persist.tile([128, 128], F32)
            nc.sync.dma_start(ident_sb[:], ident[:])
            onesb_sb = persist.tile([128, 1], BF16)
            nc.sync.dma_start(onesb_sb[:], ones_bf[:])
            onesr_sb = persist.tile([1, 128], F32R)
            nc.sync.dma_start(onesr_sb[:], ones_fr[:])

            for b in range(B):
                t0 = b * S
                # per-batch activation stores (tags shared across batches)
                qt_sb = [
                    bpool.tile([128, S], F32R, name=f"qt{qh}_b{b}", tag=f"qt{qh}")
                    for qh in range(QH_PER_CORE)
                ]
                kt_sb = bpool.tile([128, S], F32R, name=f"kt_b{b}", tag="kt")
                v_sb = bpool.tile([128, JT, 128], BF16, name=f"v_b{b}", tag="v")
                ot_sb = [
                    bpool.tile([128, S], F32R, name=f"ot{qh}_b{b}", tag=f"ot{qh}")
                    for qh in range(QH_PER_CORE)
                ]

                # ---------------- P: QKV projections ----------------
                with tc.tile_pool(name="pw", bufs=1) as pw, \
                     tc.tile_pool(name="px", bufs=3) as px, \
                     tc.tile_pool(name="pt", bufs=2) as ptmp, \
                     tc.tile_pool(name="pps", bufs=1, space="PSUM") as pps, \
                     tc.tile_pool(name="vps", bufs=2, space="PSUM") as vps:
                    wq_sb = pw.tile([128, HT, QD], F32R)
                    nc.sync.dma_start(
                        wq_sb[:], wq.rearrange("(a p) q -> p a q", p=128))

                    for tch in range(NTCH):
                        tc0 = t0 + tch * TCH
                        q_ps = [
                            pps.tile([128, TCH], F32, name=f"qps{i}", tag=f"qps{i}")
                            for i in range(QH_PER_CORE)
                        ]
                        k_ps = pps.tile([128, TCH], F32, name="kps", tag="kps")
                        v_ps = pps.tile([128, TCH], F32, name="vps0", tag="vps0")
                        for hg in range(HT // 2):
                            x_t = px.tile([128, 2, TCH], F32R, name="xs", tag="xs")
                            nc.sync.dma_start(
                                x_t[:],
                                xt[hg * 256:(hg + 1) * 256, tc0:tc0 + TCH]
                                .rearrange("(a p) t -> p a t", p=128))
                            wk_t = px.tile([128, 2, HD], F32R, name="wks", tag="wks")
                            nc.sync.dma_start(
                                wk_t[:],
                                wk[hg * 256:(hg + 1) * 256, :]
                                .rearrange("(a p) q -> p a q", p=128))
                            wv_t = px.tile([128, 2, HD], F32R, name="wvs", tag="wvs")
                            nc.sync.dma_start(
                                wv_t[:],
                                wv[hg * 256:(hg + 1) * 256, :]
                                .rearrange("(a p) q -> p a q", p=128))
                            for hi in range(2):
                                h = hg * 2 + hi
                                first, last = h == 0, h == HT - 1
                                for qd in range(QH_PER_CORE):
                                    nc.tensor.matmul(
                                        q_ps[qd][:],
                                        wq_sb[:, h, qd * 128:(qd + 1) * 128],
                                        x_t[:, hi, :], start=first, stop=last)
                                nc.tensor.matmul(k_ps[:], wk_t[:, hi, :],
                                                 x_t[:, hi, :], start=first,
                                                 stop=last)
                                nc.tensor.matmul(v_ps[:], wv_t[:, hi, :],
                                                 x_t[:, hi, :], start=first,
                                                 stop=last)
                        # RoPE (DVE, reads PSUM) → fp32r activation stores
                        csl = slice(tch * TCH, (tch + 1) * TCH)
                        for qd in range(QH_PER_CORE):
                            tA = ptmp.tile([128, TCH], F32, name="ropeA", tag="ropeA")
                            nc.vector.tensor_tensor(
                                tA[:], q_ps[qd][:], cos_sb[:, csl],
                                mybir.AluOpType.mult)
                            tBq = ptmp.tile([128, TCH], F32, name="ropeB", tag="ropeB")
                            nc.vector.tensor_tensor(
                                tBq[0:64, :], q_ps[qd][64:128, :],
                                ssin_sb[0:64, csl], mybir.AluOpType.mult)
                            nc.vector.tensor_tensor(
                                tBq[64:128, :], q_ps[qd][0:64, :],
                                ssin_sb[64:128, csl], mybir.AluOpType.mult)
                            nc.vector.tensor_tensor(
                                qt_sb[qd][:, csl], tA[:], tBq[:],
                                mybir.AluOpType.add)
                        tA = ptmp.tile([128, TCH], F32, name="ropeA", tag="ropeA")
                        nc.vector.tensor_tensor(
                            tA[:], k_ps[:], cos_sb[:, csl], mybir.AluOpType.mult)
                        tBk = ptmp.tile([128, TCH], F32, name="ropeB", tag="ropeB")
                        nc.vector.tensor_tensor(
                            tBk[0:64, :], k_ps[64:128, :], ssin_sb[0:64, csl],
                            mybir.AluOpType.mult)
                        nc.vector.tensor_tensor(
                            tBk[64:128, :], k_ps[0:64, :], ssin_sb[64:128, csl],
                            mybir.AluOpType.mult)
                        nc.vector.tensor_tensor(
                            kt_sb[:, csl], tA[:], tBk[:], mybir.AluOpType.add)
                        # V: PSUM → SBUF, then PE-transpose to token-major bf16
                        vraw = ptmp.tile([128, TCH], F32, name="vraw", tag="vraw")
                        nc.scalar.copy(vraw[:], v_ps[:])
                        for tt in range(TCH // 128):
                            vt_ps = vps.tile([128, 128], F32, name="vtp", tag="vtp")
                            nc.tensor.transpose(
                                vt_ps[:], vraw[:, tt * 128:(tt + 1) * 128],
                                ident_sb[:])
                            nc.vector.tensor_copy(
                                v_sb[:, tch * 4 + tt, :], vt_ps[:])

                # ---------------- A: attention ----------------
                with tc.tile_pool(name="ap", bufs=2) as apool, \
                     tc.tile_pool(name="an", bufs=2) as anorm, \
                     tc.tile_pool(name="sps", bufs=2, space="PSUM") as sps, \
                     tc.tile_pool(name="ops", bufs=2, space="PSUM") as ops_, \
                     tc.tile_pool(name="dps", bufs=2, space="PSUM") as dps:
                    for qh in range(QH_PER_CORE):
                        # software pipeline: scores(i+1) issued before PV(i)
                        stage = []  # (ich, p_sb, den_ps)
                        for ich in range(NTCH + 1):
                            if ich < NTCH:
                                isl = slice(ich * TCH, (ich + 1) * TCH)
                                p_sb = apool.tile([128, JT, TCH], BF16,
                                                  name="ptil", tag="ptil")
                                for jt in range(JT):
                                    st_ps = sps.tile([128, TCH], F32,
                                                     name="st", tag="st")
                                    nc.tensor.matmul(
                                        st_ps[:],
                                        kt_sb[:, jt * 128:(jt + 1) * 128],
                                        qt_sb[qh][:, isl],
                                        start=True, stop=True)
                                    nc.scalar.activation(
                                        p_sb[:, jt, :], st_ps[:],
                                        mybir.ActivationFunctionType.Exp,
                                        scale=SCALE)
                                stage.append((ich, p_sb))
                            if (ich >= 1 and stage) or ich == NTCH:
                                cich, p_sb = stage.pop(0)
                                isl = slice(cich * TCH, (cich + 1) * TCH)
                                # softmax denominator on DVE: 3D tree over
                                # j-tiles, then partition tree to one row
                                t8 = anorm.tile([128, 8, TCH], F32, name="t8",
                                                tag="t8", bufs=1)
                                nc.vector.tensor_tensor(
                                    t8[:], p_sb[:, 0:8, :], p_sb[:, 8:16, :],
                                    mybir.AluOpType.add)
                                t4 = anorm.tile([128, 4, TCH], F32, name="t4",
                                                tag="t4", bufs=1)
                                nc.vector.tensor_tensor(
                                    t4[:], t8[:, 0:4, :], t8[:, 4:8, :],
                                    mybir.AluOpType.add)
                                t2 = anorm.tile([128, 2, TCH], F32, name="t2",
                                                tag="t2", bufs=1)
                                nc.vector.tensor_tensor(
                                    t2[:], t4[:, 0:2, :], t4[:, 2:4, :],
                                    mybir.AluOpType.add)
                                t1 = anorm.tile([128, TCH], BF16, name="t1",
                                                tag="t1", bufs=2)
                                nc.vector.tensor_tensor(
                                    t1[:], t2[:, 0, :], t2[:, 1, :],
                                    mybir.AluOpType.add)
                                den_ps = dps.tile([1, TCH], F32, name="den",
                                                  tag="den")
                                nc.tensor.matmul(den_ps[:], onesb_sb[:], t1[:],
                                                 start=True, stop=True)
                                rec = anorm.tile([1, TCH], F32R, name="rec",
                                                 tag="rec")
                                nc.vector.reciprocal(rec[:], den_ps[:])
                                bc_ps = dps.tile([128, TCH], F32, name="bc",
                                                 tag="bc")
                                nc.tensor.matmul(bc_ps[:], onesr_sb[:], rec[:],
                                                 start=True, stop=True)
                                bc_sb = anorm.tile([128, TCH], F32, name="bcs",
                                                   tag="bcs")
                                nc.scalar.copy(bc_sb[:], bc_ps[:])
                                o_ps = ops_.tile([128, TCH], F32, name="ops0",
                                                 tag="ops0")
                                for jt in range(JT):
                                    nc.tensor.matmul(
                                        o_ps[:], v_sb[:, jt, :], p_sb[:, jt, :],
                                        start=(jt == 0), stop=(jt == JT - 1))
                                nc.vector.tensor_tensor(
                                    ot_sb[qh][:, isl], o_ps[:], bc_sb[:],
                                    mybir.AluOpType.mult)

                # ---------------- W: o-proj partial ----------------
                with tc.tile_pool(name="wo", bufs=2) as wop, \
                     tc.tile_pool(name="oc", bufs=4) as ocp, \
                     tc.tile_pool(name="wps", bufs=4, space="PSUM") as wps:
                    for hch in range(H // TCH):
                        hsl = slice(hch * TCH, (hch + 1) * TCH)
                        wo_t = wop.tile([128, QH_PER_CORE, TCH], F32R,
                                        name="wot", tag="wot")
                        nc.sync.dma_start(
                            wo_t[:],
                            wo[:, hsl].rearrange("(a p) hh -> p a hh", p=128))
                        for tt in range(S // 128):
                            o_ps = wps.tile([128, TCH], F32, name="wops",
                                            tag="wops")
                            for od in range(QH_PER_CORE):
                                nc.tensor.matmul(
                                    o_ps[:],
                                    ot_sb[od][:, tt * 128:(tt + 1) * 128],
                                    wo_t[:, od, :],
                                    start=(od == 0), stop=(od == QH_PER_CORE - 1))
                            o_sb = ocp.tile([128, TCH], F32, name="osb",
                                            tag="osb")
                            nc.vector.tensor_copy(o_sb[:], o_ps[:])
                            nc.sync.dma_start(
                                out[t0 + tt * 128:t0 + (tt + 1) * 128, hsl],
                                o_sb[:])
    nc.finalize()
    return nc


_NC_CACHE = None


def _get_nc():
    global _NC_CACHE
    if _NC_CACHE is None:
        _NC_CACHE = _build_nc()
    return _NC_CACHE


def _host_tables():
    inv = 1.0 / (ROPE_BASE ** (np.arange(0, HD, 2, dtype=np.float64) / HD))
    t = np.arange(S, dtype=np.float64)
    freqs = np.outer(t, inv)                      # [S, 64]
    emb = np.concatenate([freqs, freqs], axis=1)  # [S, 128]
    cos = np.cos(emb).astype(np.float32).T.copy()   # [128, S]
    sin = np.sin(emb).astype(np.float32).T.copy()
    ssin = sin.copy()
    ssin[0:64, :] *= -1.0
    return np.ascontiguousarray(cos), np.ascontiguousarray(ssin)


def kernel(hidden_states, Wq, Wk, Wv, Wo, trace=False):
    hs = np.asarray(hidden_states, dtype=np.float32)
    Wq = np.asarray(Wq, dtype=np.float32)
    Wk = np.asarray(Wk, dtype=np.float32)
    Wv = np.asarray(Wv, dtype=np.float32)
    Wo = np.asarray(Wo, dtype=np.float32)

    import ml_dtypes
    xt = np.ascontiguousarray(hs.reshape(T, H).T)          # [H, T]
    cos, ssin = _host_tables()
    ident = np.eye(128, dtype=np.float32)
    ones_bf = np.ones((128, 1), dtype=ml_dtypes.bfloat16)
    ones_fr = np.ones((1, 128), dtype=np.float32)

    in_maps = []
    for c in range(NCORES):
        in_maps.append({
            "xt": xt,
            "wq": np.ascontiguousarray(Wq[c * QD:(c + 1) * QD, :].T),
            "wk": np.ascontiguousarray(Wk[c * HD:(c + 1) * HD, :].T),
            "wv": np.ascontiguousarray(Wv[c * HD:(c + 1) * HD, :].T),
            "wo": np.ascontiguousarray(Wo[:, c * QD:(c + 1) * QD].T),
            "cosx": cos,
            "ssin": ssin,
            "ident": ident,
            "ones_bf": ones_bf,
            "ones_fr": ones_fr,
        })

    nc = _get_nc()
    res = run_bass_kernel_spmd(nc, in_maps, list(range(NCORES)), trace=trace)
    acc = np.zeros((T, H), dtype=np.float32)
    for c in range(NCORES):
        acc += res.results[c]["out_part"]
    out = acc.reshape(B, S, H)
    if trace:
        return out, res
    return out


# revision 12
# speedup vs baseline: 1.0545x; 1.0436x over previous
#!/usr/bin/env python3
"""GroupedQueryAttention Trainium2 kernel, tensor-parallel over heads on 8
NeuronCores.

Reference model: B=2, S=2048, H=4096, NH=32 query heads, NKV=8 kv heads,
HD=128, RoPE base 5e5, softmax attention, o-proj.

Sharding: core c owns kv head c and query heads 4c..4c+3 (groups stay
aligned).  Wq/Wo sharded by query head, Wk/Wv by kv head.  Each core
computes a rank-512 slice of the o-proj contraction; the host sums the 8
partial outputs (the all-reduce of row-parallel TP done on host at gather
time).

On-core dataflow (per batch b):
  P: QKV projections.  X^T streamed h-major; Wq/Wk/Wv slices resident.
     PSUM accumulates over 32 h-tiles; RoPE applied on DVE straight out of
     PSUM; V^T transposed back to token-major via PE transposes (bf16).
  A: attention per query head.  Scores computed transposed (j on
     partitions) so softmax denominators come from a ones-matmul; exp on
     ACT (fused 1/sqrt(128) scale) writes bf16 P-tilde; P@V accumulates in
     PSUM over j-tiles; normalization multiplies by a PE-broadcast
     reciprocal on the way to SBUF.
  W: o-proj partial.  O^T slices are the stationary operand directly; Wo^T
     streamed in 1 MB chunks.

All matmul inputs are float32r (full-rate PE path) except P-tilde/V which
are bf16.  Weights/activations never leave fp32 precision elsewhere.
"""
import sys

for _p in ("/opt/trn_rl_repo",):
    if _p not in sys.path:
        sys.path.insert(0, _p)

import numpy as np

import concourse.bacc as bacc
import concourse.mybir as mybir
from concourse import tile
from concourse.bass_utils import run_bass_kernel_spmd

B, S, H = 2, 2048, 4096
NH, NKV, HD = 32, 8, 128
NCORES = 8
QH_PER_CORE = NH // NCORES          # 4 query heads / core
QD = QH_PER_CORE * HD               # 512 q dims / core
ROPE_BASE = 500000.0
T = B * S                           # 4096 tokens
TCH = 512                           # token chunk (proj N, attn i-chunk)
NTCH = S // TCH                     # 4 chunks per batch
HT = H // 128                       # 32 h-tiles
JT = S // 128                       # 16 j-tiles per batch
SCALE = 1.0 / np.sqrt(HD)

F32 = mybir.dt.float32
F32R = mybir.dt.float32r
BF16 = mybir.dt.bfloat16


def _build_nc():
    nc = bacc.Bacc("TRN2", target_bir_lowering=False, debug=False)
    xt = nc.dram_tensor("xt", [H, T], F32R, kind="ExternalInput").ap()
    wq = nc.dram_tensor("wq", [H, QD], F32R, kind="ExternalInput").ap()
    wk = nc.dram_tensor("wk", [H, HD], F32R, kind="ExternalInput").ap()
    wv = nc.dram_tensor("wv", [H, HD], F32R, kind="ExternalInput").ap()
    wo = nc.dram_tensor("wo", [QD, H], F32R, kind="ExternalInput").ap()
    cosx = nc.dram_tensor("cosx", [HD, S], F32, kind="ExternalInput").ap()
    ssin = nc.dram_tensor("ssin", [HD, S], F32, kind="ExternalInput").ap()
    ident = nc.dram_tensor("ident", [128, 128], F32, kind="ExternalInput").ap()
    ones_bf = nc.dram_tensor("ones_bf", [128, 1], BF16, kind="ExternalInput").ap()
    ones_fr = nc.dram_tensor("ones_fr", [1, 128], F32R, kind="ExternalInput").ap()
    out = nc.dram_tensor("out_part", [T, H], F32, kind="ExternalOutput").ap()

    with tile.TileContext(nc) as tc, \
         nc.allow_low_precision(reason="fp32r matmul inputs; bf16 attn probs"):
        with tc.tile_pool(name="persist", bufs=1) as persist, \
             tc.tile_pool(name="batch", bufs=1) as bpool:
            cos_sb = persist.tile([HD, S], F32)
            nc.sync.dma_start(cos_sb[:], cosx[:])
            ssin_sb = persist.tile([HD, S], F32)
            nc.sync.dma_start(ssin_sb[:], ssin[:])
            ident_sb = persist.tile([128, 128], F32)
            nc.sync.dma_start(ident_sb[:], ident[:])
            onesb_sb = persist.tile([128, 1], BF16)
            nc.sync.dma_start(onesb_sb[:], ones_bf[:])
            onesr_sb = persist.tile([1, 128], F32R)
            nc.sync.dma_start(onesr_sb[:], ones_fr[:])

            for b in range(B):
                t0 = b * S
                # per-batch activation stores (tags shared across batches)
                qt_sb = [
                    bpool.tile([128, S], F32R, name=f"qt{qh}_b{b}", tag=f"qt{qh}")
                    for qh in range(QH_PER_CORE)
                ]
                kt_sb = bpool.tile([128, S], F32R, name=f"kt_b{b}", tag="kt")
                v_sb = bpool.tile([128, JT, 128], BF16, name=f"v_b{b}", tag="v")
                ot_sb = [
                    bpool.tile([128, S], F32R, name=f"ot{qh}_b{b}", tag=f"ot{qh}")
                    for qh in range(QH_PER_CORE)
                ]

                # ---------------- P: QKV projections ----------------
                with tc.tile_pool(name="pw", bufs=1) as pw, \
                     tc.tile_pool(name="px", bufs=3) as px, \
                     tc.tile_pool(name="pt", bufs=2) as ptmp, \
                     tc.tile_pool(name="pps", bufs=1, space="PSUM") as pps, \
                     tc.tile_pool(name="vps", bufs=2, space="PSUM") as vps:
                    wq_sb = pw.tile([128, HT, QD], F32R)
                    nc.sync.dma_start(
                        wq_sb[:], wq.rearrange("(a p) q -> p a q", p=128))

                    for tch in range(NTCH):
                        tc0 = t0 + tch * TCH
                        q_ps = [
                            pps.tile([128, TCH], F32, name=f"qps{i}", tag=f"qps{i}")
                            for i in range(QH_PER_CORE)
                        ]
                        k_ps = pps.tile([128, TCH], F32, name="kps", tag="kps")
                        v_ps = pps.tile([128, TCH], F32, name="vps0", tag="vps0")
                        for hg in range(HT // 2):
                            x_t = px.tile([128, 2, TCH], F32R, name="xs", tag="xs")
                            nc.sync.dma_start(
                                x_t[:],
                                xt[hg * 256:(hg + 1) * 256, tc0:tc0 + TCH]
                                .rearrange("(a p) t -> p a t", p=128))
                            wk_t = px.tile([128, 2, HD], F32R, name="wks", tag="wks")
                            nc.sync.dma_start(
                                wk_t[:],
                                wk[hg * 256:(hg + 1) * 256, :]
                                .rearrange("(a p) q -> p a q", p=128))
                            wv_t = px.tile([128, 2, HD], F32R, name="wvs", tag="wvs")
                            nc.sync.dma_start(
                                wv_t[:],
                                wv[hg * 256:(hg + 1) * 256, :]
                                .rearrange("(a p) q -> p a q", p=128))
                            for hi in range(2):
                                h = hg * 2 + hi
                                first, last = h == 0, h == HT - 1
                                for qd in range(QH_PER_CORE):
                                    nc.tensor.matmul(
                                        q_ps[qd][:],
                                        wq_sb[:, h, qd * 128:(qd + 1) * 128],
                                        x_t[:, hi, :], start=first, stop=last)
                                nc.tensor.matmul(k_ps[:], wk_t[:, hi, :],
                                                 x_t[:, hi, :], start=first,
                                                 stop=last)
                                nc.tensor.matmul(v_ps[:], wv_t[:, hi, :],
                                                 x_t[:, hi, :], start=first,
                                                 stop=last)
                        # RoPE (DVE, reads PSUM) → fp32r activation stores
                        csl = slice(tch * TCH, (tch + 1) * TCH)
                        for qd in range(QH_PER_CORE):
                            tA = ptmp.tile([128, TCH], F32, name="ropeA", tag="ropeA")
                            nc.vector.tensor_tensor(
                                tA[:], q_ps[qd][:], cos_sb[:, csl],
                                mybir.AluOpType.mult)
                            tBq = ptmp.tile([128, TCH], F32, name="ropeB", tag="ropeB")
                            nc.vector.tensor_tensor(
                                tBq[0:64, :], q_ps[qd][64:128, :],
                                ssin_sb[0:64, csl], mybir.AluOpType.mult)
                            nc.vector.tensor_tensor(
                                tBq[64:128, :], q_ps[qd][0:64, :],
                                ssin_sb[64:128, csl], mybir.AluOpType.mult)
                            nc.vector.tensor_tensor(
                                qt_sb[qd][:, csl], tA[:], tBq[:],
                                mybir.AluOpType.add)
                        tA = ptmp.tile([128, TCH], F32, name="ropeA", tag="ropeA")
                        nc.vector.tensor_tensor(
                            tA[:], k_ps[:], cos_sb[:, csl], mybir.AluOpType.mult)
                        tBk = ptmp.tile([128, TCH], F32, name="ropeB", tag="ropeB")
                        nc.vector.tensor_tensor(
                            tBk[0:64, :], k_ps[64:128, :], ssin_sb[0:64, csl],
                            mybir.AluOpType.mult)
                        nc.vector.tensor_tensor(
                            tBk[64:128, :], k_ps[0:64, :], ssin_sb[64:128, csl],
                            mybir.AluOpType.mult)
                        nc.vector.tensor_tensor(
                            kt_sb[:, csl], tA[:], tBk[:], mybir.AluOpType.add)
                        # V: PSUM → SBUF, then PE-transpose to token-major bf16
                        vraw = ptmp.tile([128, TCH], F32, name="vraw", tag="vraw")
                        nc.scalar.copy(vraw[:], v_ps[:])
                        for tt in range(TCH // 128):
                            vt_ps = vps.tile([128, 128], F32, name="vtp", tag="vtp")
                            nc.tensor.transpose(
                                vt_ps[:], vraw[:, tt * 128:(tt + 1) * 128],
                                ident_sb[:])
                            nc.vector.tensor_copy(
                                v_sb[:, tch * 4 + tt, :], vt_ps[:])

                # ---------------- A: attention ----------------
                with tc.tile_pool(name="ap", bufs=2) as apool, \
                     tc.tile_pool(name="an", bufs=2) as anorm, \
                     tc.tile_pool(name="sps", bufs=2, space="PSUM") as sps, \
                     tc.tile_pool(name="ops", bufs=2, space="PSUM") as ops_, \
                     tc.tile_pool(name="dps", bufs=2, space="PSUM") as dps:
                    for qh in range(QH_PER_CORE):
                        # software pipeline: scores(i+1) issued before PV(i)
                        stage = []  # (ich, p_sb, den_ps)
                        for ich in range(NTCH + 1):
                            if ich < NTCH:
                                isl = slice(ich * TCH, (ich + 1) * TCH)
                                p_sb = apool.tile([128, JT, TCH], BF16,
                                                  name="ptil", tag="ptil")
                                for jt in range(JT):
                                    st_ps = sps.tile([128, TCH], F32,
                                                     name="st", tag="st")
                                    nc.tensor.matmul(
                                        st_ps[:],
                                        kt_sb[:, jt * 128:(jt + 1) * 128],
                                        qt_sb[qh][:, isl],
                                        start=True, stop=True)
                                    nc.scalar.activation(
                                        p_sb[:, jt, :], st_ps[:],
                                        mybir.ActivationFunctionType.Exp,
                                        scale=SCALE)
                                stage.append((ich, p_sb))
                            if (ich >= 1 and stage) or ich == NTCH:
                                cich, p_sb = stage.pop(0)
                                isl = slice(cich * TCH, (cich + 1) * TCH)
                                # softmax denominator on DVE: 3D tree over
                                # j-tiles, then partition tree to one row
                                t8 = anorm.tile([128, 8, TCH], F32, name="t8",
                                                tag="t8", bufs=1)
                                nc.vector.tensor_tensor(
                                    t8[:], p_sb[:, 0:8, :], p_sb[:, 8:16, :],
                                    mybir.AluOpType.add)
                                t4 = anorm.tile([128, 4, TCH], F32, name="t4",
                                                tag="t4", bufs=1)
                                nc.vector.tensor_tensor(
                                    t4[:], t8[:, 0:4, :], t8[:, 4:8, :],
                                    mybir.AluOpType.add)
                                t2 = anorm.tile([128, 2, TCH], F32, name="t2",
                                                tag="t2", bufs=1)
                                nc.vector.tensor_tensor(
                                    t2[:], t4[:, 0:2, :], t4[:, 2:4, :],
                                    mybir.AluOpType.add)
                                t1 = anorm.tile([128, TCH], BF16, name="t1",
                                                tag="t1", bufs=2)
                                nc.vector.tensor_tensor(
                                    t1[:], t2[:, 0, :], t2[:, 1, :],
                                    mybir.AluOpType.add)
                                den_ps = dps.tile([1, TCH], F32, name="den",
                                                  tag="den")
                                nc.tensor.matmul(den_ps[:], onesb_sb[:], t1[:],
                                                 start=True, stop=True)
                                rec = anorm.tile([1, TCH], F32R, name="rec",
                                                 tag="rec")
                                nc.vector.reciprocal(rec[:], den_ps[:])
                                bc_ps = dps.tile([128, TCH], F32, name="bc",
                                                 tag="bc")
                                nc.tensor.matmul(bc_ps[:], onesr_sb[:], rec[:],
                                                 start=True, stop=True)
                                bc_sb = anorm.tile([128, TCH], F32, name="bcs",
                                                   tag="bcs")
                                nc.scalar.copy(bc_sb[:], bc_ps[:])
                                o_ps = ops_.tile([128, TCH], F32, name="ops0",
                                                 tag="ops0")
                                for jt in range(JT):
                                    nc.tensor.matmul(
                                        o_ps[:], v_sb[:, jt, :], p_sb[:, jt, :],
                                        start=(jt == 0), stop=(jt == JT - 1))
                                nc.vector.tensor_tensor(
                                    ot_sb[qh][:, isl], o_ps[:], bc_sb[:],
                                    mybir.AluOpType.mult)

                # ---------------- W: o-proj partial ----------------
                with tc.tile_pool(name="wo", bufs=2) as wop, \
                     tc.tile_pool(name="oc", bufs=4) as ocp, \
                     tc.tile_pool(name="wps", bufs=4, space="PSUM") as wps:
                    for hch in range(H // TCH):
                        hsl = slice(hch * TCH, (hch + 1) * TCH)
                        wo_t = wop.tile([128, QH_PER_CORE, TCH], F32R,
                                        name="wot", tag="wot")
                        nc.scalar.dma_start(
                            wo_t[:],
                            wo[:, hsl].rearrange("(a p) hh -> p a hh", p=128))
                        for ttg in range(4):
                            o_sb = ocp.tile([128, 4, TCH], F32, name="osb",
                                            tag="osb")
                            for a in range(4):
                                tt = ttg * 4 + a
                                o_ps = wps.tile([128, TCH], F32, name="wops",
                                                tag="wops")
                                for od in range(QH_PER_CORE):
                                    nc.tensor.matmul(
                                        o_ps[:],
                                        ot_sb[od][:, tt * 128:(tt + 1) * 128],
                                        wo_t[:, od, :],
                                        start=(od == 0), stop=(od == QH_PER_CORE - 1))
                                nc.vector.tensor_copy(o_sb[:, a, :], o_ps[:])
                            nc.scalar.dma_start(
                                out[t0 + ttg * 512:t0 + (ttg + 1) * 512, hsl]
                                .rearrange("(a p) hh -> p a hh", p=128),
                                o_sb[:])
    nc.finalize()
    return nc


_NC_CACHE = None


def _get_nc():
    global _NC_CACHE
    if _NC_CACHE is None:
        _NC_CACHE = _build_nc()
    return _NC_CACHE


def _host_tables():
    inv = 1.0 / (ROPE_BASE ** (np.arange(0, HD, 2, dtype=np.float64) / HD))
    t = np.arange(S, dtype=np.float64)
    freqs = np.outer(t, inv)                      # [S, 64]
    emb = np.concatenate([freqs, freqs], axis=1)  # [S, 128]
    cos = np.cos(emb).astype(np.float32).T.copy()   # [128, S]
    sin = np.sin(emb).astype(np.float32).T.copy()
    ssin = sin.copy()
    ssin[0:64, :] *= -1.0
    return np.ascontiguousarray(cos), np.ascontiguousarray(ssin)


def kernel(hidden_states, Wq, Wk, Wv, Wo, trace=False):
    hs = np.asarray(hidden_states, dtype=np.float32)
    Wq = np.asarray(Wq, dtype=np.float32)
    Wk = np.asarray(Wk, dtype=np.float32)
    Wv = np.asarray(Wv, dtype=np.float32)
    Wo = np.asarray(Wo, dtype=np.float32)

    import ml_dtypes
    xt = np.ascontiguousarray(hs.reshape(T, H).T)          # [H, T]
    cos, ssin = _host_tables()
    ident = np.eye(128, dtype=np.float32)
    ones_bf = np.ones((128, 1), dtype=ml_dtypes.bfloat16)
    ones_fr = np.ones((1, 128), dtype=np.float32)

    in_maps = []
    for c in range(NCORES):
        in_maps.append({
            "xt": xt,
            "wq": np.ascontiguousarray(Wq[c * QD:(c + 1) * QD, :].T),
            "wk": np.ascontiguousarray(Wk[c * HD:(c + 1) * HD, :].T),
            "wv": np.ascontiguousarray(Wv[c * HD:(c + 1) * HD, :].T),
            "wo": np.ascontiguousarray(Wo[:, c * QD:(c + 1) * QD].T),
            "cosx": cos,
            "ssin": ssin,
            "ident": ident,
            "ones_bf": ones_bf,
            "ones_fr": ones_fr,
        })

    nc = _get_nc()
    res = run_bass_kernel_spmd(nc, in_maps, list(range(NCORES)), trace=trace)
    acc = np.zeros((T, H), dtype=np.float32)
    for c in range(NCORES):
        acc += res.results[c]["out_part"]
    out = acc.reshape(B, S, H)
    if trace:
        return out, res
    return out
